# revision 18
# baseline (speedup 1.0000x reference)
"""Causal self-attention on 8 trn2 NeuronCores.

Sharding: core = (batch b, head-group g), b in 0..3, g in 0..1.
Each core handles its batch and 8 heads (512 of 1024 embed dims):
  qT/kT projections stored transposed [e', s] (e' on partitions), bf16
  V stored [s, e'] bf16 with a ones-column per head (softmax denom)
  S^T[k, q] = K_h Q_h^T   (head pair packed in PE row groups, concurrent)
  P^T = exp(S^T / 8) fused over both heads: one ACTIVATE reads the
        2-bank PSUM score pair [128, 2, w] -> bf16 pt tile
  causal zeroing of P^T via one gpsimd affine_select per diag kt (both heads)
  att'^T[d, q] accumulates over kt into a 2-bank PSUM pair (row 64 = l)
  att_n^T = att'^T * (1/l)  (reciprocal straight off PSUM, gpsimd bcast)
  out_partial = att_n^T.T @ Wo[rows_g, :]; host sums the two g-partials.

Everything is bf16 (1 cycle/row on PE like fp32r, but FWL halves
LDWEIGHTS and DMA/SBUF traffic halves).  x is DMA'd once and cached in
SBUF; projections, attention, and the output projection are emitted
interleaved from a fill queue so the PE never idles long enough for the
HAM clock gate to re-throttle, and the ScalarE exp stream (the attention
pacer) starts early and stays busy.
"""
import collections
import math
import sys

if "/opt/trn_rl_repo" not in sys.path:
    sys.path.insert(0, "/opt/trn_rl_repo")

import ml_dtypes
import numpy as np

import concourse.bacc as bacc
import concourse.mybir as mybir
import concourse.tile as tile
from concourse.bass_utils import run_bass_kernel_spmd

S = 2048          # sequence length
E = 1024          # embed dim
G = 512           # per-core head-group width (8 heads x 64)
HD = 64           # head dim
NH = 8            # heads per core
EC = E // 128     # 8 e-chunks
ST = S // 128     # 16 s-tiles
SB = S // 512     # 4 s-blocks
F32 = mybir.dt.float32
BF16 = mybir.dt.bfloat16
EXP = mybir.ActivationFunctionType.Exp
GE = mybir.AluOpType.is_ge
NPBF16 = ml_dtypes.bfloat16

_CACHE = {}


def _emit(nc, tc):
    xT = nc.declare_dram_parameter("xT", [E, S], BF16, isOutput=False)
    # wq/wk packed on host as [pair, partition, ec, col]
    wq = nc.declare_dram_parameter("wq", [4, 128, EC, 128], BF16,
                                   isOutput=False)
    wk = nc.declare_dram_parameter("wk", [4, 128, EC, 128], BF16,
                                   isOutput=False)
    wv = nc.declare_dram_parameter("wv", [E, G], BF16, isOutput=False)
    wo = nc.declare_dram_parameter("wo", [G, E], BF16, isOutput=False)
    c_ones = nc.declare_dram_parameter("c_ones", [128, NH], BF16,
                                       isOutput=False)
    out = nc.declare_dram_parameter("out", [S, E], F32, isOutput=True)

    # ---- pools (allocated once; released in LIFO order at the end) ----
    persist = tc.alloc_tile_pool(name="persist", bufs=1, side="right")
    sc_pool = tc.alloc_tile_pool(name="scp", bufs=2, space="PSUM")   # 4 banks
    at_pool = tc.alloc_tile_pool(name="atp", bufs=1, space="PSUM")   # 2 banks
    pp_pool = tc.alloc_tile_pool(name="ppp", bufs=2, space="PSUM")   # 2 banks
    pt_pool = tc.alloc_tile_pool(name="ptp", bufs=3)
    smalls = tc.alloc_tile_pool(name="smalls", bufs=2)
    ostage = tc.alloc_tile_pool(name="ostage", bufs=4)

    # ---- persistent SBUF state ----
    xsb = [persist.tile([128, S], BF16, name=f"x{ec}", tag=f"x{ec}")
           for ec in range(EC)]
    qT = [persist.tile([128, S], BF16, name=f"qT{c}", tag=f"qT{c}")
          for c in range(4)]
    kT = [persist.tile([128, S], BF16, name=f"kT{c}", tag=f"kT{c}")
          for c in range(4)]
    vP = [persist.tile([128, NH, HD + 1], BF16, name=f"vP{st}",
                       tag=f"vP{st}") for st in range(ST)]
    att_n = [persist.tile([128, S], BF16, name=f"attn{c}", tag=f"attn{c}")
             for c in range(4)]
    wq_sb = [persist.tile([128, EC, 128], BF16, name=f"wq{c}", tag=f"wq{c}")
             for c in range(4)]
    wk_sb = [persist.tile([128, EC, 128], BF16, name=f"wk{c}", tag=f"wk{c}")
             for c in range(4)]
    wv_sb = [persist.tile([128, G], BF16, name=f"wv{ec}", tag=f"wv{ec}")
             for ec in range(EC)]
    wo_sb = [persist.tile([128, E], BF16, name=f"wo{c}", tag=f"wo{c}")
             for c in range(4)]
    ones_sb = persist.tile([128, NH], BF16, name="ones_sb", tag="ones_sb")

    # ---- input DMAs, in the order the PE will need them ----
    # wq0 + per-ec x (sb0,sb1) first so the first qk unit can start ~2us in
    nc.sync.dma_start(out=wq_sb[0], in_=wq[0])
    for ec in range(EC):
        nc.sync.dma_start(out=xsb[ec][:, 0:1024],
                          in_=xT[ec * 128:(ec + 1) * 128, 0:1024])
    nc.sync.dma_start(out=wk_sb[0], in_=wk[0])
    for ec in range(EC):
        nc.sync.dma_start(out=wv_sb[ec],
                          in_=wv[ec * 128:(ec + 1) * 128, :])
    nc.sync.dma_start(out=ones_sb, in_=c_ones[:, :])
    for c in range(1, 4):
        nc.sync.dma_start(out=wq_sb[c], in_=wq[c])
        nc.sync.dma_start(out=wk_sb[c], in_=wk[c])
    for ec in range(EC):
        nc.sync.dma_start(out=xsb[ec][:, 1024:2048],
                          in_=xT[ec * 128:(ec + 1) * 128, 1024:2048])
    for c in range(4):
        nc.sync.dma_start(out=wo_sb[c],
                          in_=wo[c * 128:(c + 1) * 128, :])

    # ---- PE warmup spin ----
    # Dummy matmuls on a never-initialized SBUF tile (no DMA dependency):
    # keeps the PE busy through the ~8us DMA dead-zone at kernel start so
    # the HAM clock-gate reaches 8/8 before real work arrives.
    warm_sb = persist.tile([128, 640], BF16, name="warm_sb", tag="warm_sb")
    nc.vector.memset(warm_sb, 0.0)
    warm_ps = [pp_pool.tile([128, 512], F32, name=f"warm{i}", tag="pp")
               for i in range(2)]
    for i in range(22):
        nc.tensor.matmul(warm_ps[i % 2], lhsT=warm_sb[:, 0:128],
                         rhs=warm_sb[:, 128:640],
                         start=(i < 2), stop=(i >= 20),
                         skip_group_check=True)

    # ---- fill queue machinery ----
    fill = collections.deque()   # items: (gid, closure)
    group_left = {}
    state = {"kt_left": 160}

    def push_group(gid, units):
        group_left[gid] = len(units)
        for u in units:
            fill.append((gid, u))

    def pump(n):
        c = 0
        while c < n and fill:
            gid, u = fill.popleft()
            u()
            group_left[gid] -= 1
            c += 1



    def drain_until(gids):
        want = [g for g in gids if group_left.get(g, 0) > 0]
        while want:
            gid, u = fill.popleft()
            u()
            group_left[gid] -= 1
            want = [g for g in gids if group_left.get(g, 0) > 0]

    # ---- work-unit generators ----
    def qk_group_units(c, w, sbp):
        """Q or K projection for pair c, s-blocks (2*sbp, 2*sbp+1).
        One LDWEIGHTS per ec serves two matmuls (two s-blocks)."""
        wsb = wq_sb[c] if w == "q" else wk_sb[c]
        dest = qT[c] if w == "q" else kT[c]
        sbA, sbB = 2 * sbp, 2 * sbp + 1
        h = {}
        units = []

        def mk(ec):
            def mmu():
                if ec == 0:
                    h["pA"] = pp_pool.tile([128, 512], F32, name="ppA",
                                           tag="pp")
                    h["pB"] = pp_pool.tile([128, 512], F32, name="ppB",
                                           tag="pp")
                lhs = wsb[:, ec, :]
                nc.tensor.matmul(h["pA"], lhsT=lhs,
                                 rhs=xsb[ec][:, sbA * 512:(sbA + 1) * 512],
                                 start=(ec == 0), stop=(ec == EC - 1),
                                 skip_group_check=True)
                nc.tensor.matmul(h["pB"], lhsT=lhs,
                                 rhs=xsb[ec][:, sbB * 512:(sbB + 1) * 512],
                                 start=(ec == 0), stop=(ec == EC - 1),
                                 skip_group_check=True)
            return mmu
        for ec in range(EC):
            units.append((mk(ec), 530.0))
        units.append((lambda: nc.vector.tensor_copy(
            dest[:, sbA * 512:(sbA + 1) * 512], h["pA"]), 0.0))
        units.append((lambda: nc.vector.tensor_copy(
            dest[:, sbB * 512:(sbB + 1) * 512], h["pB"]), 0.0))
        return units

    def v_group_units(st):
        """V projection for s-tile st (all 8 heads), plus the ones column."""
        h = {}
        units = []

        def mk(ec):
            def mmu():
                if ec == 0:
                    h["ps"] = pp_pool.tile([128, 512], F32, name="ppV",
                                           tag="pp")
                nc.tensor.matmul(h["ps"],
                                 lhsT=xsb[ec][:, st * 128:(st + 1) * 128],
                                 rhs=wv_sb[ec],
                                 start=(ec == 0), stop=(ec == EC - 1),
                                 skip_group_check=True)
            return mmu
        for ec in range(EC):
            units.append((mk(ec), 315.0))

        def cp():
            nc.vector.tensor_copy(
                vP[st][:, :, 0:HD],
                h["ps"].rearrange("p (h d) -> p h d", h=NH))
            nc.vector.tensor_copy(vP[st][:, :, HD], ones_sb)
        units.append((cp, 0.0))
        return units

    def outproj_units(qb):
        """Output projection for the 4 s-tiles of q-block qb.
        One LDWEIGHTS (att_n slice) serves two matmuls (two e-halves)."""
        units = []
        for st in range(4 * qb, 4 * qb + 4):
            h = {}

            def mk(c, st=st, h=h):
                def mmu():
                    if c == 0:
                        h["pA"] = pp_pool.tile([128, 512], F32, name="poA",
                                               tag="pp")
                        h["pB"] = pp_pool.tile([128, 512], F32, name="poB",
                                               tag="pp")
                    lhs = att_n[c][:, st * 128:(st + 1) * 128]
                    nc.tensor.matmul(h["pA"], lhsT=lhs,
                                     rhs=wo_sb[c][:, 0:512],
                                     start=(c == 0), stop=(c == 3),
                                     skip_group_check=True)
                    nc.tensor.matmul(h["pB"], lhsT=lhs,
                                     rhs=wo_sb[c][:, 512:1024],
                                     start=(c == 0), stop=(c == 3),
                                     skip_group_check=True)
                return mmu
            for c in range(4):
                units.append((mk(c), 530.0))

            def cpu(st=st, h=h):
                for eb, ps in ((0, h["pA"]), (1, h["pB"])):
                    o = ostage.tile([128, 512], F32, name="o_sb", tag="o")
                    nc.vector.tensor_copy(o, ps)
                    nc.sync.dma_start(
                        out=out[st * 128:(st + 1) * 128,
                                eb * 512:(eb + 1) * 512],
                        in_=o)
            units.append((cpu, 0.0))
        return units

    # ---- build the fill queue ----
    push_group(("qk", 0, "q", 0), qk_group_units(0, "q", 0))
    push_group(("qk", 0, "k", 0), qk_group_units(0, "k", 0))
    for st in range(0, 4):
        push_group(("v", st), v_group_units(st))
    for c in range(1, 4):
        push_group(("qk", c, "q", 0), qk_group_units(c, "q", 0))
        push_group(("qk", c, "k", 0), qk_group_units(c, "k", 0))
    for st in range(4, 8):
        push_group(("v", st), v_group_units(st))
    for st in range(8, 12):
        push_group(("v", st), v_group_units(st))
    for c in range(4):
        push_group(("qk", c, "q", 1), qk_group_units(c, "q", 1))
        push_group(("qk", c, "k", 1), qk_group_units(c, "k", 1))
    for st in range(12, 16):
        push_group(("v", st), v_group_units(st))

    # ---- attention block ----
    def attention_block(c, qb):
        nkt = 4 * qb + 4
        att = at_pool.tile([128, 1024], F32, name="att", tag="att")
        att3 = att.rearrange("p (u q) -> p u q", u=2)
        pend = None   # (kt, cs, pt) whose attnV is deferred one kt


        def attv(kt, cs, pt):
            for u in range(2):
                nc.tensor.matmul(
                    att3[0:HD + 1, u, cs:512],
                    lhsT=vP[kt][:, 2 * c + u, :],
                    rhs=pt[:, u, cs:512],
                    start=(kt == 0), stop=(kt == nkt - 1),
                    skip_group_check=True)

        for kt in range(nkt):
            if kt < 4 * qb:
                cs, diag = 0, False
            else:
                cs, diag = 128 * kt - 512 * qb, True
            w = 512 - cs
            sc = sc_pool.tile([128, 1024], F32, name="sc", tag="sc")
            sc3 = sc.rearrange("p (u q) -> p u q", u=2)
            for u in range(2):
                po = u * HD
                nc.tensor.matmul(
                    sc[:, u * 512 + cs:(u + 1) * 512],
                    lhsT=kT[c][po:po + HD, kt * 128:(kt + 1) * 128],
                    rhs=qT[c][po:po + HD, qb * 512 + cs:(qb + 1) * 512],
                    start=True, stop=True, skip_group_check=True,
                    tile_position=(po, 0))
            clk["pe"] += w / 2.4 + 95.0          # concurrent score pair
            pt = pt_pool.tile([128, 2, 512], BF16, name="pt", tag="pt")
            nc.scalar.activation(pt[:, :, cs:512], sc3[:, :, cs:512], EXP,
                                 scale=0.125)
            clk["act"] = max(clk["act"], clk["pe"]) + (2 * w + 330) / 1.2
            if diag:
                # zero invalid (k > q): valid iff y - x >= 0 in-tile
                nc.gpsimd.affine_select(
                    out=pt[:, :, cs:512], in_=pt[:, :, cs:512],
                    compare_op=GE, fill=0.0,
                    base=0, channel_multiplier=-1,
                    pattern=[[0, 2], [1, w]])
            # pump fill while the PE is modeled behind the exp stream
            pump_while_behind()
            # attnV runs one kt behind: its exp/select chain gets a full
            # kt of PE work (next scores + fill) as latency cover
            if pend is not None:
                attv(*pend)
                clk["pe"] += 2 * (512 - pend[1]) / 2.4 + 160.0
            pend = (kt, cs, pt)
        attv(*pend)
        clk["pe"] += 2 * (512 - pend[1]) / 2.4 + 160.0
        # normalization: copy l (PSUM row 64) to SBUF, r = 1/l, bcast, scale
        # (l-copy must NOT go on ScalarE: it would make every later exp
        # in the ACT FIFO wait for this block's last attnV)
        l_sb = smalls.tile([1, 1024], F32, name="l_sb", tag="l")
        nc.vector.tensor_copy(l_sb, att[HD:HD + 1, :])
        r_sb = smalls.tile([1, 1024], F32, name="r_sb", tag="r")
        nc.vector.reciprocal_approx_fast(out=r_sb, in_=l_sb)
        rb0 = smalls.tile([HD, 512], F32, name="rb0", tag="rb0")
        nc.gpsimd.partition_broadcast(rb0, r_sb[:, 0:512])
        rb1 = smalls.tile([HD, 512], F32, name="rb1", tag="rb1")
        nc.gpsimd.partition_broadcast(rb1, r_sb[:, 512:1024])
        qsl = slice(qb * 512, (qb + 1) * 512)
        nc.vector.tensor_mul(att_n[c][0:HD, qsl], att3[0:HD, 0, :], rb0)
        nc.vector.tensor_mul(att_n[c][HD:2 * HD, qsl], att3[0:HD, 1, :], rb1)
        pump_while_behind(margin=1500.0)

    # ---- main schedule: rows of blocks with interleaved fill ----
    for qb in range(4):
        for c in range(4):
            need = [("v", st) for st in range(4 * qb + 4)]
            need += [("qk", c, w, sbp)
                     for w in ("q", "k") for sbp in range(qb // 2 + 1)]
            drain_until(need)
            attention_block(c, qb)
        push_group(("op", qb), outproj_units(qb))
    pump(10 ** 9)  # drain everything left (final outproj row)

    # release in LIFO order
    ostage.release()
    smalls.release()
    pt_pool.release()
    pp_pool.release()
    at_pool.release()
    sc_pool.release()
    persist.release()


def _build():
    if "nc" in _CACHE:
        return _CACHE["nc"]
    nc = bacc.Bacc()
    with tile.TileContext(nc) as tc:
        _emit(nc, tc)
    nc.compile()
    _CACHE["nc"] = nc
    return nc


def _pack_w(Wg):
    # [E, G] -> [pair c, partition p, ec, col m]:
    # out[c, p, ec, m] = Wg[ec*128 + p, c*128 + m]
    return np.ascontiguousarray(
        Wg.reshape(EC, 128, 4, 128).transpose(2, 1, 0, 3))


def _make_in_maps(inputs):
    x = np.asarray(inputs["x"], dtype=np.float32)
    Wq = np.asarray(inputs["Wq"], dtype=np.float32)
    Wk = np.asarray(inputs["Wk"], dtype=np.float32)
    Wv = np.asarray(inputs["Wv"], dtype=np.float32)
    Wo = np.asarray(inputs["Wo"], dtype=np.float32)
    in_maps = []
    for core in range(8):
        b, g = core // 2, core % 2
        cols = slice(g * G, (g + 1) * G)
        in_maps.append({
            "xT": np.ascontiguousarray(x[b].T).astype(NPBF16),
            "wq": _pack_w(Wq[:, cols]).astype(NPBF16),
            "wk": _pack_w(Wk[:, cols]).astype(NPBF16),
            "wv": np.ascontiguousarray(Wv[:, cols]).astype(NPBF16),
            "wo": np.ascontiguousarray(Wo[cols, :]).astype(NPBF16),
            "c_ones": np.ones((128, NH), dtype=NPBF16),
        })
    return in_maps


def kernel(x, Wq, Wk, Wv, Wo):
    nc = _build()
    in_maps = _make_in_maps(dict(x=x, Wq=Wq, Wk=Wk, Wv=Wv, Wo=Wo))
    res = run_bass_kernel_spmd(nc, in_maps, core_ids=list(range(8)))
    out = np.zeros((4, S, E), dtype=np.float32)
    for core in range(8):
        out[core // 2] += res.results[core]["out"]
    return out


if __name__ == "__main__":
    rng = np.random.default_rng(0)
    x = rng.standard_normal((4, S, E), dtype=np.float32)
    sc = 1.0 / np.sqrt(E)
    Wq = rng.standard_normal((E, E), dtype=np.float32) * sc
    Wk = rng.standard_normal((E, E), dtype=np.float32) * sc
    Wv = rng.standard_normal((E, E), dtype=np.float32) * sc
    Wo = rng.standard_normal((E, E), dtype=np.float32) * sc
    o = kernel(x, Wq, Wk, Wv, Wo)
    print("out", o.shape, o.dtype, np.abs(o).mean())


# revision 19
# speedup vs baseline: 1.1288x; 1.1288x over previous
"""Causal self-attention on 8 trn2 NeuronCores.

Sharding: core = (batch b, head-group g), b in 0..3, g in 0..1.
Each core handles its batch and 8 heads (512 of 1024 embed dims):
  qT/kT projections stored transposed [e', s] (e' on partitions), bf16
  V stored [s, e'] bf16 with a ones-column per head (softmax denom)
  S^T[k, q] = K_h Q_h^T   (head pair packed in PE row groups, concurrent)
  P^T = exp(S^T / 8) fused over both heads: one ACTIVATE reads the
        2-bank PSUM score pair [128, 2, w] -> bf16 pt tile
  causal zeroing of P^T via one gpsimd affine_select per diag kt (both heads)
  att'^T[d, q] accumulates over kt into a 2-bank PSUM pair (row 64 = l)
  att_n^T = att'^T * (1/l)  (reciprocal straight off PSUM, gpsimd bcast)
  out_partial = att_n^T.T @ Wo[rows_g, :]; host sums the two g-partials.

Everything is bf16 (1 cycle/row on PE like fp32r, but FWL halves
LDWEIGHTS and DMA/SBUF traffic halves).  x is DMA'd once and cached in
SBUF; projections, attention, and the output projection are emitted
interleaved from a fill queue so the PE never idles long enough for the
HAM clock gate to re-throttle, and the ScalarE exp stream (the attention
pacer) starts early and stays busy.
"""
import collections
import math
import sys

if "/opt/trn_rl_repo" not in sys.path:
    sys.path.insert(0, "/opt/trn_rl_repo")

import ml_dtypes
import numpy as np

import concourse.bacc as bacc
import concourse.mybir as mybir
import concourse.tile as tile
from concourse.bass_utils import run_bass_kernel_spmd

S = 2048          # sequence length
E = 1024          # embed dim
G = 512           # per-core head-group width (8 heads x 64)
HD = 64           # head dim
NH = 8            # heads per core
EC = E // 128     # 8 e-chunks
ST = S // 128     # 16 s-tiles
SB = S // 512     # 4 s-blocks
F32 = mybir.dt.float32
BF16 = mybir.dt.bfloat16
EXP = mybir.ActivationFunctionType.Exp
GE = mybir.AluOpType.is_ge
NPBF16 = ml_dtypes.bfloat16

_CACHE = {}


def _emit(nc, tc):
    xT = nc.declare_dram_parameter("xT", [E, S], BF16, isOutput=False)
    # wq/wk packed on host as [pair, partition, ec, col]
    wq = nc.declare_dram_parameter("wq", [4, 128, EC, 128], BF16,
                                   isOutput=False)
    wk = nc.declare_dram_parameter("wk", [4, 128, EC, 128], BF16,
                                   isOutput=False)
    wv = nc.declare_dram_parameter("wv", [E, G], BF16, isOutput=False)
    wo = nc.declare_dram_parameter("wo", [G, E], BF16, isOutput=False)
    c_ones = nc.declare_dram_parameter("c_ones", [128, NH], BF16,
                                       isOutput=False)
    out = nc.declare_dram_parameter("out", [S, E], F32, isOutput=True)

    # ---- pools (allocated once; released in LIFO order at the end) ----
    persist = tc.alloc_tile_pool(name="persist", bufs=1, side="right")
    sc_pool = tc.alloc_tile_pool(name="scp", bufs=2, space="PSUM")   # 4 banks
    at_pool = tc.alloc_tile_pool(name="atp", bufs=1, space="PSUM")   # 2 banks
    pp_pool = tc.alloc_tile_pool(name="ppp", bufs=2, space="PSUM")   # 2 banks
    pt_pool = tc.alloc_tile_pool(name="ptp", bufs=3)
    smalls = tc.alloc_tile_pool(name="smalls", bufs=2)
    ostage = tc.alloc_tile_pool(name="ostage", bufs=4)

    # ---- persistent SBUF state ----
    xsb = [persist.tile([128, S], BF16, name=f"x{ec}", tag=f"x{ec}")
           for ec in range(EC)]
    qT = [persist.tile([128, S], BF16, name=f"qT{c}", tag=f"qT{c}")
          for c in range(4)]
    kT = [persist.tile([128, S], BF16, name=f"kT{c}", tag=f"kT{c}")
          for c in range(4)]
    vP = [persist.tile([128, NH, HD + 1], BF16, name=f"vP{st}",
                       tag=f"vP{st}") for st in range(ST)]
    att_n = [persist.tile([128, S], BF16, name=f"attn{c}", tag=f"attn{c}")
             for c in range(4)]
    wq_sb = [persist.tile([128, EC, 128], BF16, name=f"wq{c}", tag=f"wq{c}")
             for c in range(4)]
    wk_sb = [persist.tile([128, EC, 128], BF16, name=f"wk{c}", tag=f"wk{c}")
             for c in range(4)]
    wv_sb = [persist.tile([128, G], BF16, name=f"wv{ec}", tag=f"wv{ec}")
             for ec in range(EC)]
    wo_sb = [persist.tile([128, E], BF16, name=f"wo{c}", tag=f"wo{c}")
             for c in range(4)]
    ones_sb = persist.tile([128, NH], BF16, name="ones_sb", tag="ones_sb")

    # ---- input DMAs, in the order the PE will need them ----
    # wq0 + per-ec x (sb0,sb1) first so the first qk unit can start ~2us in
    nc.sync.dma_start(out=wq_sb[0], in_=wq[0])
    for ec in range(EC):
        nc.sync.dma_start(out=xsb[ec][:, 0:1024],
                          in_=xT[ec * 128:(ec + 1) * 128, 0:1024])
    nc.sync.dma_start(out=wk_sb[0], in_=wk[0])
    for ec in range(EC):
        nc.sync.dma_start(out=wv_sb[ec],
                          in_=wv[ec * 128:(ec + 1) * 128, :])
    nc.sync.dma_start(out=ones_sb, in_=c_ones[:, :])
    for c in range(1, 4):
        nc.sync.dma_start(out=wq_sb[c], in_=wq[c])
        nc.sync.dma_start(out=wk_sb[c], in_=wk[c])
    for ec in range(EC):
        nc.sync.dma_start(out=xsb[ec][:, 1024:2048],
                          in_=xT[ec * 128:(ec + 1) * 128, 1024:2048])
    for c in range(4):
        nc.sync.dma_start(out=wo_sb[c],
                          in_=wo[c * 128:(c + 1) * 128, :])

    # ---- PE warmup spin ----
    # Dummy matmuls on a never-initialized SBUF tile (no DMA dependency):
    # keeps the PE busy through the ~8us DMA dead-zone at kernel start so
    # the HAM clock-gate reaches 8/8 before real work arrives.
    warm_sb = persist.tile([128, 640], BF16, name="warm_sb", tag="warm_sb")
    nc.vector.memset(warm_sb, 0.0)
    warm_ps = [pp_pool.tile([128, 512], F32, name=f"warm{i}", tag="pp")
               for i in range(2)]
    for i in range(22):
        nc.tensor.matmul(warm_ps[i % 2], lhsT=warm_sb[:, 0:128],
                         rhs=warm_sb[:, 128:640],
                         start=(i < 2), stop=(i >= 20),
                         skip_group_check=True)

    # ---- fill queue machinery ----
    fill = collections.deque()   # items: (gid, closure)
    group_left = {}
    state = {"kt_left": 160}

    def push_group(gid, units):
        group_left[gid] = len(units)
        for u in units:
            fill.append((gid, u))

    def pump(n):
        c = 0
        while c < n and fill:
            gid, u = fill.popleft()
            u()
            group_left[gid] -= 1
            c += 1



    def drain_until(gids):
        want = [g for g in gids if group_left.get(g, 0) > 0]
        while want:
            gid, u = fill.popleft()
            u()
            group_left[gid] -= 1
            want = [g for g in gids if group_left.get(g, 0) > 0]

    # ---- work-unit generators ----
    def qk_group_units(c, w, sbp):
        """Q or K projection for pair c, s-blocks (2*sbp, 2*sbp+1).
        One LDWEIGHTS per ec serves two matmuls (two s-blocks)."""
        wsb = wq_sb[c] if w == "q" else wk_sb[c]
        dest = qT[c] if w == "q" else kT[c]
        sbA, sbB = 2 * sbp, 2 * sbp + 1
        h = {}
        units = []

        def mk(ec):
            def mmu():
                if ec == 0:
                    h["pA"] = pp_pool.tile([128, 512], F32, name="ppA",
                                           tag="pp")
                    h["pB"] = pp_pool.tile([128, 512], F32, name="ppB",
                                           tag="pp")
                lhs = wsb[:, ec, :]
                nc.tensor.matmul(h["pA"], lhsT=lhs,
                                 rhs=xsb[ec][:, sbA * 512:(sbA + 1) * 512],
                                 start=(ec == 0), stop=(ec == EC - 1),
                                 skip_group_check=True)
                nc.tensor.matmul(h["pB"], lhsT=lhs,
                                 rhs=xsb[ec][:, sbB * 512:(sbB + 1) * 512],
                                 start=(ec == 0), stop=(ec == EC - 1),
                                 skip_group_check=True)
            return mmu
        for ec in range(EC):
            units.append((mk(ec), 530.0))
        units.append((lambda: nc.vector.tensor_copy(
            dest[:, sbA * 512:(sbA + 1) * 512], h["pA"]), 0.0))
        units.append((lambda: nc.vector.tensor_copy(
            dest[:, sbB * 512:(sbB + 1) * 512], h["pB"]), 0.0))
        return units

    def v_group_units(st):
        """V projection for s-tile st (all 8 heads), plus the ones column."""
        h = {}
        units = []

        def mk(ec):
            def mmu():
                if ec == 0:
                    h["ps"] = pp_pool.tile([128, 512], F32, name="ppV",
                                           tag="pp")
                nc.tensor.matmul(h["ps"],
                                 lhsT=xsb[ec][:, st * 128:(st + 1) * 128],
                                 rhs=wv_sb[ec],
                                 start=(ec == 0), stop=(ec == EC - 1),
                                 skip_group_check=True)
            return mmu
        for ec in range(EC):
            units.append((mk(ec), 315.0))

        def cp():
            nc.vector.tensor_copy(
                vP[st][:, :, 0:HD],
                h["ps"].rearrange("p (h d) -> p h d", h=NH))
            nc.vector.tensor_copy(vP[st][:, :, HD], ones_sb)
        units.append((cp, 0.0))
        return units

    def outproj_units(qb):
        """Output projection for the 4 s-tiles of q-block qb.
        One LDWEIGHTS (att_n slice) serves two matmuls (two e-halves)."""
        units = []
        for st in range(4 * qb, 4 * qb + 4):
            h = {}

            def mk(c, st=st, h=h):
                def mmu():
                    if c == 0:
                        h["pA"] = pp_pool.tile([128, 512], F32, name="poA",
                                               tag="pp")
                        h["pB"] = pp_pool.tile([128, 512], F32, name="poB",
                                               tag="pp")
                    lhs = att_n[c][:, st * 128:(st + 1) * 128]
                    nc.tensor.matmul(h["pA"], lhsT=lhs,
                                     rhs=wo_sb[c][:, 0:512],
                                     start=(c == 0), stop=(c == 3),
                                     skip_group_check=True)
                    nc.tensor.matmul(h["pB"], lhsT=lhs,
                                     rhs=wo_sb[c][:, 512:1024],
                                     start=(c == 0), stop=(c == 3),
                                     skip_group_check=True)
                return mmu
            for c in range(4):
                units.append((mk(c), 530.0))

            def cpu(st=st, h=h):
                for eb, ps in ((0, h["pA"]), (1, h["pB"])):
                    o = ostage.tile([128, 512], F32, name="o_sb", tag="o")
                    nc.vector.tensor_copy(o, ps)
                    nc.sync.dma_start(
                        out=out[st * 128:(st + 1) * 128,
                                eb * 512:(eb + 1) * 512],
                        in_=o)
            units.append((cpu, 0.0))
        return units

    # ---- build the fill queue ----
    push_group(("qk", 0, "q", 0), qk_group_units(0, "q", 0))
    push_group(("qk", 0, "k", 0), qk_group_units(0, "k", 0))
    for st in range(0, 4):
        push_group(("v", st), v_group_units(st))
    for c in range(1, 4):
        push_group(("qk", c, "q", 0), qk_group_units(c, "q", 0))
        push_group(("qk", c, "k", 0), qk_group_units(c, "k", 0))
    for st in range(4, 8):
        push_group(("v", st), v_group_units(st))
    for st in range(8, 12):
        push_group(("v", st), v_group_units(st))
    for c in range(4):
        push_group(("qk", c, "q", 1), qk_group_units(c, "q", 1))
        push_group(("qk", c, "k", 1), qk_group_units(c, "k", 1))
    for st in range(12, 16):
        push_group(("v", st), v_group_units(st))

    # ---- attention block ----
    def attention_block(c, qb):
        nkt = 4 * qb + 4
        att = at_pool.tile([128, 1024], F32, name="att", tag="att")
        att3 = att.rearrange("p (u q) -> p u q", u=2)
        pend = None   # (kt, cs, pt) whose attnV is deferred one kt
        chain_prev = 0.0   # modeled completion of pend's exp(+select)


        def attv(kt, cs, pt):
            for u in range(2):
                nc.tensor.matmul(
                    att3[0:HD + 1, u, cs:512],
                    lhsT=vP[kt][:, 2 * c + u, :],
                    rhs=pt[:, u, cs:512],
                    start=(kt == 0), stop=(kt == nkt - 1),
                    skip_group_check=True)

        for kt in range(nkt):
            if kt < 4 * qb:
                cs, diag = 0, False
            else:
                cs, diag = 128 * kt - 512 * qb, True
            w = 512 - cs
            sc = sc_pool.tile([128, 1024], F32, name="sc", tag="sc")
            sc3 = sc.rearrange("p (u q) -> p u q", u=2)
            for u in range(2):
                po = u * HD
                nc.tensor.matmul(
                    sc[:, u * 512 + cs:(u + 1) * 512],
                    lhsT=kT[c][po:po + HD, kt * 128:(kt + 1) * 128],
                    rhs=qT[c][po:po + HD, qb * 512 + cs:(qb + 1) * 512],
                    start=True, stop=True, skip_group_check=True,
                    tile_position=(po, 0))
            clk["pe"] += w / 2.4 + 95.0          # concurrent score pair
            pt = pt_pool.tile([128, 2, 512], BF16, name="pt", tag="pt")
            nc.scalar.activation(pt[:, :, cs:512], sc3[:, :, cs:512], EXP,
                                 scale=0.125)
            clk["act"] = max(clk["act"], clk["pe"]) + (2 * w + 330) / 1.2
            chain = clk["act"]
            if diag:
                # zero invalid (k > q): valid iff y - x >= 0 in-tile
                nc.gpsimd.affine_select(
                    out=pt[:, :, cs:512], in_=pt[:, :, cs:512],
                    compare_op=GE, fill=0.0,
                    base=0, channel_multiplier=-1,
                    pattern=[[0, 2], [1, w]])
                chain += (2 * w) / 1.5   # gpsimd select on the chain
            # pump fill until the PE is modeled past the point where the
            # DEFERRED attnV's exp/select chain completes
            while fill and clk["pe"] < chain_prev:
                pump(1)
            # attnV runs one kt behind: its exp/select chain gets a full
            # kt of PE work (next scores + fill) as latency cover
            if pend is not None:
                attv(*pend)
                clk["pe"] += 2 * (512 - pend[1]) / 2.4 + 160.0
            pend = (kt, cs, pt)
            chain_prev = chain
        attv(*pend)
        clk["pe"] += 2 * (512 - pend[1]) / 2.4 + 160.0
        # normalization: copy l (PSUM row 64) to SBUF, r = 1/l, bcast, scale
        # (l-copy must NOT go on ScalarE: it would make every later exp
        # in the ACT FIFO wait for this block's last attnV)
        l_sb = smalls.tile([1, 1024], F32, name="l_sb", tag="l")
        nc.vector.tensor_copy(l_sb, att[HD:HD + 1, :])
        r_sb = smalls.tile([1, 1024], F32, name="r_sb", tag="r")
        nc.vector.reciprocal_approx_fast(out=r_sb, in_=l_sb)
        rb0 = smalls.tile([HD, 512], F32, name="rb0", tag="rb0")
        nc.gpsimd.partition_broadcast(rb0, r_sb[:, 0:512])
        rb1 = smalls.tile([HD, 512], F32, name="rb1", tag="rb1")
        nc.gpsimd.partition_broadcast(rb1, r_sb[:, 512:1024])
        qsl = slice(qb * 512, (qb + 1) * 512)
        nc.vector.tensor_mul(att_n[c][0:HD, qsl], att3[0:HD, 0, :], rb0)
        nc.vector.tensor_mul(att_n[c][HD:2 * HD, qsl], att3[0:HD, 1, :], rb1)
        pump_while_behind(margin=1500.0)

    # ---- main schedule: rows of blocks with interleaved fill ----
    for qb in range(4):
        for c in range(4):
            need = [("v", st) for st in range(4 * qb + 4)]
            need += [("qk", c, w, sbp)
                     for w in ("q", "k") for sbp in range(qb // 2 + 1)]
            drain_until(need)
            attention_block(c, qb)
        push_group(("op", qb), outproj_units(qb))
    pump(10 ** 9)  # drain everything left (final outproj row)

    # release in LIFO order
    ostage.release()
    smalls.release()
    pt_pool.release()
    pp_pool.release()
    at_pool.release()
    sc_pool.release()
    persist.release()


def _build():
    if "nc" in _CACHE:
        return _CACHE["nc"]
    nc = bacc.Bacc()
    with tile.TileContext(nc) as tc:
        _emit(nc, tc)
    nc.compile()
    _CACHE["nc"] = nc
    return nc


def _pack_w(Wg):
    # [E, G] -> [pair c, partition p, ec, col m]:
    # out[c, p, ec, m] = Wg[ec*128 + p, c*128 + m]
    return np.ascontiguousarray(
        Wg.reshape(EC, 128, 4, 128).transpose(2, 1, 0, 3))


def _make_in_maps(inputs):
    x = np.asarray(inputs["x"], dtype=np.float32)
    Wq = np.asarray(inputs["Wq"], dtype=np.float32)
    Wk = np.asarray(inputs["Wk"], dtype=np.float32)
    Wv = np.asarray(inputs["Wv"], dtype=np.float32)
    Wo = np.asarray(inputs["Wo"], dtype=np.float32)
    in_maps = []
    for core in range(8):
        b, g = core // 2, core % 2
        cols = slice(g * G, (g + 1) * G)
        in_maps.append({
            "xT": np.ascontiguousarray(x[b].T).astype(NPBF16),
            "wq": _pack_w(Wq[:, cols]).astype(NPBF16),
            "wk": _pack_w(Wk[:, cols]).astype(NPBF16),
            "wv": np.ascontiguousarray(Wv[:, cols]).astype(NPBF16),
            "wo": np.ascontiguousarray(Wo[cols, :]).astype(NPBF16),
            "c_ones": np.ones((128, NH), dtype=NPBF16),
        })
    return in_maps


def kernel(x, Wq, Wk, Wv, Wo):
    nc = _build()
    in_maps = _make_in_maps(dict(x=x, Wq=Wq, Wk=Wk, Wv=Wv, Wo=Wo))
    res = run_bass_kernel_spmd(nc, in_maps, core_ids=list(range(8)))
    out = np.zeros((4, S, E), dtype=np.float32)
    for core in range(8):
        out[core // 2] += res.results[core]["out"]
    return out


if __name__ == "__main__":
    rng = np.random.default_rng(0)
    x = rng.standard_normal((4, S, E), dtype=np.float32)
    sc = 1.0 / np.sqrt(E)
    Wq = rng.standard_normal((E, E), dtype=np.float32) * sc
    Wk = rng.standard_normal((E, E), dtype=np.float32) * sc
    Wv = rng.standard_normal((E, E), dtype=np.float32) * sc
    Wo = rng.standard_normal((E, E), dtype=np.float32) * sc
    o = kernel(x, Wq, Wk, Wv, Wo)
    print("out", o.shape, o.dtype, np.abs(o).mean())
